# revision 11
# baseline (speedup 1.0000x reference)
"""Multi-head attention (B=2, S=2048, D=1024, H=16, Hd=64) on 8 Trainium2
NeuronCores.

Sharding: 8 cores = (batch 2) x (head-half 2) x (q-half 2).
Core (b, hh, qh) computes, for batch b, heads hh*8..hh*8+8 and query rows
qh*1024..qh*1024+1024, the partial output

    outp = (softmax-attention of its heads restricted to its q rows) @ Wo_part.T
           + bo_part

and the host sums the two head-half partials per (b, qh) block.  bo is fed as
zeros to the hh==1 cores so the bias is counted once.

Device-side layouts (host pre-transposes so every matmul is a natural
lhsT.T @ rhs with the contraction dim on SBUF partitions):
  xT    [D, S]      x[b].T
  wqT/wkT/wvT [D, 512]  W.T column slice for this head-half
  woT   [512, D]    Wo.T row slice for this head-half
  maskT [S, 1024]   mask[b,0].T column slice for this q-half (int32)

Pipeline per core:
  1. qT = (wqT.T @ xT-cols)  [512, 1024],  kT [512, 2048], V [2048, 512]
     all fp32r (full PE speed, ~1e-4 matmul error).
  2. Per head h, per s_k tile i: scoresT tile [128, 1024] = kT_h_i.T @ qT_h
     (K=64; head pairs land on PE row-groups 0-63/64-127 and run
     concurrently), exp on ScalarE (scale=1/8) -> bf16, mask multiply on
     VectorE (bf16, 2x mode), then attnV accumulation
     out_ps [128, 512] += V_aug_i.T @ expm  where V_aug has 64 ones
     columns so PSUM rows 64..127 all hold Z = sum(expm); reciprocal of
     those rows gives 1/Z already replicated across partitions.
  3. out partial [1024, 1024] = out_cT.T @ woT (+ bo broadcast), DMA out.

No collectives; the only cross-core step is the host-side partial sum.
"""

import sys

if "/opt/trn_rl_repo" not in sys.path:
    sys.path.insert(0, "/opt/trn_rl_repo")

import numpy as np

B, S, D = 2, 2048, 1024
H, HD = 16, 64
NCORES = 8
HPC = 8  # heads per core
DPC = HPC * HD  # 512 head dims per core
SQC = S // 2  # 1024 q rows per core
KT = D // 128  # 8 contraction tiles
NSK = S // 128  # 16 s_k tiles
NDB = DPC // 128  # 4 d-blocks of the per-core head dims

_CACHE = {}


def _build():
    import concourse.bacc as bacc
    import concourse.mybir as mybir
    import concourse.tile as tile

    F32 = mybir.dt.float32
    F32R = mybir.dt.float32r
    BF16 = mybir.dt.bfloat16
    I32 = mybir.dt.int32
    MULT = mybir.AluOpType.mult
    ADD = mybir.AluOpType.add
    EQ = mybir.AluOpType.is_equal
    EXP = mybir.ActivationFunctionType.Exp

    nc = bacc.Bacc("TRN2", target_bir_lowering=False, debug=False)

    xT = nc.dram_tensor("xT", [D, S], F32, kind="ExternalInput")
    wqT = nc.dram_tensor("wqT", [D, DPC], F32, kind="ExternalInput")
    wkT = nc.dram_tensor("wkT", [D, DPC], F32, kind="ExternalInput")
    wvT = nc.dram_tensor("wvT", [D, DPC], F32, kind="ExternalInput")
    woT = nc.dram_tensor("woT", [DPC, D], F32, kind="ExternalInput")
    maskT = nc.dram_tensor("maskT", [S, SQC], I32, kind="ExternalInput")
    bo = nc.dram_tensor("bo", [D], F32, kind="ExternalInput")
    outp = nc.dram_tensor("outp", [SQC, D], F32, kind="ExternalOutput")

    xT_r = xT.rearrange("(t p) s -> p t s", p=128)  # [128, KT, S]
    wqT_r = wqT.rearrange("(t p) d -> p t d", p=128)
    wkT_r = wkT.rearrange("(t p) d -> p t d", p=128)
    wvT_r = wvT.rearrange("(t p) d -> p t d", p=128)
    woT_r = woT.rearrange("(c p) d -> p c d", p=128)  # [128, NDB, D]
    maskT_r = maskT.rearrange("(i p) q -> p i q", p=128)  # [128, NSK, SQC]

    with tile.TileContext(nc) as tc:
        with tc.tile_pool(name="keep", bufs=1) as keep:
            # ---- persistent SBUF tensors --------------------------------
            qT_sb = keep.tile([128, NDB, SQC], F32R)  # 16KB/part
            kT_sb = keep.tile([128, NDB, S], F32R)  # 32KB/part
            v_aug = keep.tile([128, NSK, HPC * 128], BF16)  # 32KB/part
            out_cT = keep.tile([128, NDB, SQC], F32R)  # 16KB/part

            # ones block of V_aug (overwritten below on the V columns)
            nc.vector.memset(v_aug[:], 1.0)

            # ---- phase 1: projections (all fp32r) -----------------------
            with (
                tc.tile_pool(name="p1", bufs=1) as p1,
                tc.tile_pool(name="wslot", bufs=2) as wslot,
                tc.tile_pool(name="ps1", bufs=6, space="PSUM") as ps1,
            ):
                x_sb = p1.tile([128, KT, S], F32R)  # 64KB/part
                for t in range(KT):
                    nc.gpsimd.dma_start(out=x_sb[:, t, :], in_=xT_r[:, t, :])

                def load_w(src_r, nm):
                    w = wslot.tile([128, KT, DPC], F32R, tag="w", name=f"w_{nm}")
                    nc.gpsimd.dma_start(out=w[:], in_=src_r[:])
                    return w

                # qT [512, SQC]: lhsT = wqT tile, rhs = xT q-columns
                wq = load_w(wqT_r, "q")
                for db in range(NDB):
                    for j in range(SQC // 512):
                        ps = ps1.tile([128, 512], F32, tag="ps")
                        for t in range(KT):
                            nc.tensor.matmul(
                                ps[:],
                                wq[:, t, db * 128 : (db + 1) * 128],
                                x_sb[:, t, j * 512 : (j + 1) * 512],
                                start=(t == 0),
                                stop=(t == KT - 1),
                            )
                        nc.any.tensor_copy(
                            qT_sb[:, db, j * 512 : (j + 1) * 512], ps[:]
                        )

                # kT [512, S]
                wk = load_w(wkT_r, "k")
                for db in range(NDB):
                    for j in range(S // 512):
                        ps = ps1.tile([128, 512], F32, tag="ps")
                        for t in range(KT):
                            nc.tensor.matmul(
                                ps[:],
                                wk[:, t, db * 128 : (db + 1) * 128],
                                x_sb[:, t, j * 512 : (j + 1) * 512],
                                start=(t == 0),
                                stop=(t == KT - 1),
                            )
                        nc.any.tensor_copy(
                            kT_sb[:, db, j * 512 : (j + 1) * 512], ps[:]
                        )

                # V natural [S, 512]: lhsT = xT s-block (stationary), rhs = wvT
                wv = load_w(wvT_r, "v")
                for sb in range(NSK):
                    ps = ps1.tile([128, 512], F32, tag="ps")
                    for t in range(KT):
                        nc.tensor.matmul(
                            ps[:],
                            x_sb[:, t, sb * 128 : (sb + 1) * 128],
                            wv[:, t, :],
                            start=(t == 0),
                            stop=(t == KT - 1),
                        )
                    # scatter 8 heads' 64 V columns into v_aug head blocks
                    nc.any.tensor_copy(
                        v_aug[:, sb, :]
                        .rearrange("p (h c) -> p h c", h=HPC)[:, :, 0:HD],
                        ps[:].rearrange("p (h c) -> p h c", h=HPC),
                    )

            # ---- phase 2: attention -------------------------------------
            with (
                tc.tile_pool(name="pmask", bufs=1) as pmask,
                tc.tile_pool(name="mstage", bufs=3) as mstage,
                tc.tile_pool(name="p2", bufs=3) as p2,
                tc.tile_pool(name="sc", bufs=2, space="PSUM") as scp,
                tc.tile_pool(name="op", bufs=4, space="PSUM") as opp,
            ):
                # mask conversion: int32 0/1 -> bf16 (mask==0 -> 1.0)
                mask01 = pmask.tile([128, NSK, SQC], BF16)  # 32KB/part
                for i in range(NSK):
                    for half in range(2):
                        sl = slice(half * (SQC // 2), (half + 1) * (SQC // 2))
                        mi = mstage.tile([128, SQC // 2], I32, tag="mi")
                        nc.sync.dma_start(out=mi[:], in_=maskT_r[:, i, sl])
                        nc.vector.tensor_scalar(
                            out=mask01[:, i, sl],
                            in0=mi[:],
                            scalar1=0,
                            scalar2=None,
                            op0=EQ,
                        )
                for hp in range(HPC // 2):  # head pairs
                    out_ps = [
                        [
                            opp.tile([128, 512], F32, tag="ops", name=f"ops_{hp}_{h2}_{j}")
                            for j in range(2)
                        ]
                        for h2 in range(2)
                    ]  # [h2][j]
                    for i in range(NSK):
                        for h2 in range(2):
                            h = 2 * hp + h2
                            ksl = kT_sb[
                                h2 * 64 : (h2 + 1) * 64, hp, i * 128 : (i + 1) * 128
                            ]
                            sc = scp.tile([128, 1024], F32, tag="sc")
                            for j in range(2):
                                nc.tensor.matmul(
                                    sc[:, j * 512 : (j + 1) * 512],
                                    ksl,
                                    qT_sb[
                                        h2 * 64 : (h2 + 1) * 64,
                                        hp,
                                        j * 512 : (j + 1) * 512,
                                    ],
                                    start=True,
                                    stop=True,
                                )
                            expt = p2.tile([128, 1024], BF16, tag="expt")
                            nc.scalar.activation(
                                out=expt[:], in_=sc[:], func=EXP, scale=0.125
                            )
                            expm = p2.tile([128, 1024], BF16, tag="expm")
                            nc.vector.tensor_tensor(
                                out=expm[:],
                                in0=expt[:],
                                in1=mask01[:, i, :],
                                op=MULT,
                            )
                            vsl = v_aug[:, i, h * 128 : (h + 1) * 128]
                            for j in range(2):
                                nc.tensor.matmul(
                                    out_ps[h2][j][:],
                                    vsl,
                                    expm[:, j * 512 : (j + 1) * 512],
                                    start=(i == 0),
                                    stop=(i == NSK - 1),
                                )
                    # normalize: rows 64..127 of out_ps hold Z replicated
                    for h2 in range(2):
                        for j in range(2):
                            zr = p2.tile([64, 512], F32, tag="zr")
                            nc.vector.reciprocal(
                                out=zr[:], in_=out_ps[h2][j][64:128, :]
                            )
                            nc.vector.tensor_tensor(
                                out=out_cT[
                                    h2 * 64 : (h2 + 1) * 64,
                                    hp,
                                    j * 512 : (j + 1) * 512,
                                ],
                                in0=out_ps[h2][j][0:64, :],
                                in1=zr[:],
                                op=MULT,
                            )

            # ---- phase 3: output projection -----------------------------
            with (
                tc.tile_pool(name="p3", bufs=1) as p3,
                tc.tile_pool(name="p3w", bufs=3) as p3w,
                tc.tile_pool(name="ps3", bufs=4, space="PSUM") as ps3,
            ):
                wo_sb = p3.tile([128, NDB, D], F32R)
                nc.gpsimd.dma_start(out=wo_sb[:], in_=woT_r[:])
                bo_rep = p3.tile([128, D], F32)
                nc.sync.dma_start(
                    out=bo_rep[:], in_=bo.ap()[None, :].to_broadcast((128, D))
                )

                for m in range(SQC // 128):
                    for n in range(D // 512):
                        ps = ps3.tile([128, 512], F32, tag="ps3")
                        for c in range(NDB):
                            nc.tensor.matmul(
                                ps[:],
                                out_cT[:, c, m * 128 : (m + 1) * 128],
                                wo_sb[:, c, n * 512 : (n + 1) * 512],
                                start=(c == 0),
                                stop=(c == NDB - 1),
                            )
                        ob = p3w.tile([128, 512], F32, tag="ob")
                        nc.vector.tensor_tensor(
                            out=ob[:],
                            in0=ps[:],
                            in1=bo_rep[:, n * 512 : (n + 1) * 512],
                            op=ADD,
                        )
                        nc.sync.dma_start(
                            out=outp[m * 128 : (m + 1) * 128, n * 512 : (n + 1) * 512],
                            in_=ob[:],
                        )

    nc.compile()
    return nc


def _get_nc():
    if "nc" not in _CACHE:
        _CACHE["nc"] = _build()
    return _CACHE["nc"]


def _prep_inputs(x, mask, Wq, Wk, Wv, Wo, bo):
    """Build the 8 per-core input maps."""
    x = np.asarray(x, dtype=np.float32)
    mask = np.asarray(mask, dtype=np.int32)
    bo = np.asarray(bo, dtype=np.float32)
    wqT = np.ascontiguousarray(np.asarray(Wq, np.float32).T)
    wkT = np.ascontiguousarray(np.asarray(Wk, np.float32).T)
    wvT = np.ascontiguousarray(np.asarray(Wv, np.float32).T)
    woT = np.ascontiguousarray(np.asarray(Wo, np.float32).T)
    bz = np.zeros_like(bo)

    # The SPMD program always reads q activations from xT columns 0..SQC,
    # so qh==1 cores get xT rolled by -SQC along s (and maskT rows rolled
    # identically).  Attention sums over s_k, so a consistent permutation
    # of the k/V order (with the mask following it) leaves the result
    # unchanged.
    xTs = [np.ascontiguousarray(x[b].T) for b in range(B)]
    xTs_r = [np.ascontiguousarray(np.roll(t, -SQC, axis=1)) for t in xTs]
    maskTs = [np.ascontiguousarray(mask[b, 0].T) for b in range(B)]
    maskTs_r = [np.roll(t, -SQC, axis=0) for t in maskTs]

    in_maps = []
    for c in range(NCORES):
        b, hh, qh = c >> 2, (c >> 1) & 1, c & 1
        doff = hh * DPC
        qoff = qh * SQC
        mT = maskTs[b] if qh == 0 else maskTs_r[b]
        in_maps.append(
            {
                "xT": xTs[b] if qh == 0 else xTs_r[b],
                "wqT": np.ascontiguousarray(wqT[:, doff : doff + DPC]),
                "wkT": np.ascontiguousarray(wkT[:, doff : doff + DPC]),
                "wvT": np.ascontiguousarray(wvT[:, doff : doff + DPC]),
                "woT": np.ascontiguousarray(woT[doff : doff + DPC, :]),
                "maskT": np.ascontiguousarray(mT[:, qoff : qoff + SQC]),
                "bo": bo if hh == 0 else bz,
            }
        )
    return in_maps


def run(inputs: dict, trace: bool = False):
    """Run the kernel; returns (full_output, BassKernelResults)."""
    from concourse.bass_utils import run_bass_kernel_spmd

    nc = _get_nc()
    in_maps = _prep_inputs(**inputs)
    res = run_bass_kernel_spmd(
        nc, in_maps, core_ids=list(range(NCORES)), trace=trace
    )
    out = np.empty((B, S, D), dtype=np.float32)
    for b in range(B):
        for qh in range(2):
            c0 = (b << 2) | (0 << 1) | qh
            c1 = (b << 2) | (1 << 1) | qh
            out[b, qh * SQC : (qh + 1) * SQC, :] = (
                res.results[c0]["outp"] + res.results[c1]["outp"]
            )
    return out, res


def kernel(**inputs) -> np.ndarray:
    out, _ = run(inputs, trace=False)
    return out


# revision 12
# speedup vs baseline: 1.1891x; 1.1891x over previous
"""Multi-head attention (B=2, S=2048, D=1024, H=16, Hd=64) on 8 Trainium2
NeuronCores.

Sharding: 8 cores = (batch 2) x (head-half 2) x (q-half 2).
Core (b, hh, qh) computes, for batch b, heads hh*8..hh*8+8 and query rows
qh*1024..qh*1024+1024, the partial output

    outp = (softmax-attention of its heads restricted to its q rows) @ Wo_part.T
           + bo_part

and the host sums the two head-half partials per (b, qh) block.  bo is fed as
zeros to the hh==1 cores so the bias is counted once.

Device-side layouts (host pre-transposes so every matmul is a natural
lhsT.T @ rhs with the contraction dim on SBUF partitions):
  xT    [D, S]      x[b].T
  wqT/wkT/wvT [D, 512]  W.T column slice for this head-half
  woT   [512, D]    Wo.T row slice for this head-half
  maskT [S, 1024]   mask[b,0].T column slice for this q-half (int32)

Pipeline per core:
  1. qT = (wqT.T @ xT-cols)  [512, 1024],  kT [512, 2048], V [2048, 512]
     all fp32r (full PE speed, ~1e-4 matmul error).
  2. Per head h, per s_k tile i: scoresT tile [128, 1024] = kT_h_i.T @ qT_h
     (K=64; head pairs land on PE row-groups 0-63/64-127 and run
     concurrently), exp on ScalarE (scale=1/8) -> bf16, mask multiply on
     VectorE (bf16, 2x mode), then attnV accumulation
     out_ps [128, 512] += V_aug_i.T @ expm  where V_aug has 64 ones
     columns so PSUM rows 64..127 all hold Z = sum(expm); reciprocal of
     those rows gives 1/Z already replicated across partitions.
  3. out partial [1024, 1024] = out_cT.T @ woT (+ bo broadcast), DMA out.

No collectives; the only cross-core step is the host-side partial sum.
"""

import sys

if "/opt/trn_rl_repo" not in sys.path:
    sys.path.insert(0, "/opt/trn_rl_repo")

import numpy as np

B, S, D = 2, 2048, 1024
H, HD = 16, 64
NCORES = 8
HPC = 8  # heads per core
DPC = HPC * HD  # 512 head dims per core
SQC = S // 2  # 1024 q rows per core
KT = D // 128  # 8 contraction tiles
NSK = S // 128  # 16 s_k tiles
NDB = DPC // 128  # 4 d-blocks of the per-core head dims

_CACHE = {}


def _build():
    import concourse.bacc as bacc
    import concourse.mybir as mybir
    import concourse.tile as tile

    F32 = mybir.dt.float32
    F32R = mybir.dt.float32r
    BF16 = mybir.dt.bfloat16
    I32 = mybir.dt.int32
    MULT = mybir.AluOpType.mult
    ADD = mybir.AluOpType.add
    EQ = mybir.AluOpType.is_equal
    EXP = mybir.ActivationFunctionType.Exp

    nc = bacc.Bacc("TRN2", target_bir_lowering=False, debug=False)

    xT = nc.dram_tensor("xT", [D, S], F32, kind="ExternalInput")
    wqT = nc.dram_tensor("wqT", [D, DPC], F32, kind="ExternalInput")
    wkT = nc.dram_tensor("wkT", [D, DPC], F32, kind="ExternalInput")
    wvT = nc.dram_tensor("wvT", [D, DPC], F32, kind="ExternalInput")
    woT = nc.dram_tensor("woT", [DPC, D], F32, kind="ExternalInput")
    maskT = nc.dram_tensor("maskT", [S, SQC], I32, kind="ExternalInput")
    bo = nc.dram_tensor("bo", [D], F32, kind="ExternalInput")
    outp = nc.dram_tensor("outp", [SQC, D], F32, kind="ExternalOutput")

    xT_r = xT.rearrange("(t p) s -> p t s", p=128)  # [128, KT, S]
    wqT_r = wqT.rearrange("(t p) d -> p t d", p=128)
    wkT_r = wkT.rearrange("(t p) d -> p t d", p=128)
    wvT_r = wvT.rearrange("(t p) d -> p t d", p=128)
    woT_r = woT.rearrange("(c p) d -> p c d", p=128)  # [128, NDB, D]
    maskT_r = maskT.rearrange("(i p) q -> p i q", p=128)  # [128, NSK, SQC]

    with tile.TileContext(nc) as tc:
        with tc.tile_pool(name="keep", bufs=1) as keep:
            # ---- persistent SBUF tensors --------------------------------
            qT_sb = keep.tile([128, NDB, SQC], F32R)  # 16KB/part
            kT_sb = keep.tile([128, NDB, S], F32R)  # 32KB/part
            v_aug = keep.tile([128, NSK, HPC * 128], BF16)  # 32KB/part
            out_cT = keep.tile([128, NDB, SQC], F32R)  # 16KB/part

            # ones block of V_aug (overwritten below on the V columns)
            nc.vector.memset(v_aug[:], 1.0)

            # ---- phase 1: projections (all fp32r) -----------------------
            with (
                tc.tile_pool(name="p1", bufs=1) as p1,
                tc.tile_pool(name="wslot", bufs=2) as wslot,
                tc.tile_pool(name="ps1", bufs=6, space="PSUM") as ps1,
            ):
                x_sb = p1.tile([128, KT, S], F32R)  # 64KB/part
                for t in range(KT):
                    nc.gpsimd.dma_start(out=x_sb[:, t, :], in_=xT_r[:, t, :])

                def load_w(src_r, nm):
                    w = wslot.tile([128, KT, DPC], F32R, tag="w", name=f"w_{nm}")
                    nc.gpsimd.dma_start(out=w[:], in_=src_r[:])
                    return w

                # qT [512, SQC]: lhsT = wqT tile, rhs = xT q-columns
                wq = load_w(wqT_r, "q")
                for db in range(NDB):
                    for j in range(SQC // 512):
                        ps = ps1.tile([128, 512], F32, tag="ps")
                        for t in range(KT):
                            nc.tensor.matmul(
                                ps[:],
                                wq[:, t, db * 128 : (db + 1) * 128],
                                x_sb[:, t, j * 512 : (j + 1) * 512],
                                start=(t == 0),
                                stop=(t == KT - 1),
                            )
                        nc.any.tensor_copy(
                            qT_sb[:, db, j * 512 : (j + 1) * 512], ps[:]
                        )

                # kT [512, S]
                wk = load_w(wkT_r, "k")
                for db in range(NDB):
                    for j in range(S // 512):
                        ps = ps1.tile([128, 512], F32, tag="ps")
                        for t in range(KT):
                            nc.tensor.matmul(
                                ps[:],
                                wk[:, t, db * 128 : (db + 1) * 128],
                                x_sb[:, t, j * 512 : (j + 1) * 512],
                                start=(t == 0),
                                stop=(t == KT - 1),
                            )
                        nc.any.tensor_copy(
                            kT_sb[:, db, j * 512 : (j + 1) * 512], ps[:]
                        )

                # V natural [S, 512]: lhsT = xT s-block (stationary), rhs = wvT
                wv = load_w(wvT_r, "v")
                for sb in range(NSK):
                    ps = ps1.tile([128, 512], F32, tag="ps")
                    for t in range(KT):
                        nc.tensor.matmul(
                            ps[:],
                            x_sb[:, t, sb * 128 : (sb + 1) * 128],
                            wv[:, t, :],
                            start=(t == 0),
                            stop=(t == KT - 1),
                        )
                    # scatter 8 heads' 64 V columns into v_aug head blocks
                    nc.any.tensor_copy(
                        v_aug[:, sb, :]
                        .rearrange("p (h c) -> p h c", h=HPC)[:, :, 0:HD],
                        ps[:].rearrange("p (h c) -> p h c", h=HPC),
                    )

            # ---- phase 2: attention -------------------------------------
            with (
                tc.tile_pool(name="pmask", bufs=1) as pmask,
                tc.tile_pool(name="mstage", bufs=3) as mstage,
                tc.tile_pool(name="p2", bufs=3) as p2,
                tc.tile_pool(name="pexpm", bufs=4) as pexpm,
                tc.tile_pool(name="sc", bufs=2, space="PSUM") as scp,
                tc.tile_pool(name="op", bufs=4, space="PSUM") as opp,
            ):
                # mask conversion: int32 0/1 -> bf16 (mask==0 -> 1.0)
                mask01 = pmask.tile([128, NSK, SQC], BF16)  # 32KB/part
                for i in range(NSK):
                    for half in range(2):
                        sl = slice(half * (SQC // 2), (half + 1) * (SQC // 2))
                        mi = mstage.tile([128, SQC // 2], I32, tag="mi")
                        nc.sync.dma_start(out=mi[:], in_=maskT_r[:, i, sl])
                        nc.vector.tensor_scalar(
                            out=mask01[:, i, sl],
                            in0=mi[:],
                            scalar1=0,
                            scalar2=None,
                            op0=EQ,
                        )

                def emit_front(hp, i, expm_q):
                    # scores + exp + mask for both heads of the pair
                    for h2 in range(2):
                        ksl = kT_sb[
                            h2 * 64 : (h2 + 1) * 64, hp, i * 128 : (i + 1) * 128
                        ]
                        sc = scp.tile([128, 1024], F32, tag="sc", name=f"sc_{hp}_{i}_{h2}")
                        for j in range(2):
                            nc.tensor.matmul(
                                sc[:, j * 512 : (j + 1) * 512],
                                ksl,
                                qT_sb[
                                    h2 * 64 : (h2 + 1) * 64,
                                    hp,
                                    j * 512 : (j + 1) * 512,
                                ],
                                start=True,
                                stop=True,
                            )
                        expt = p2.tile([128, 1024], BF16, tag="expt")
                        nc.scalar.activation(
                            out=expt[:], in_=sc[:], func=EXP, scale=0.125
                        )
                        expm = pexpm.tile(
                            [128, 1024], BF16, tag="expm", name=f"expm_{hp}_{i}_{h2}"
                        )
                        nc.vector.tensor_tensor(
                            out=expm[:], in0=expt[:], in1=mask01[:, i, :], op=MULT
                        )
                        expm_q[(i, h2)] = expm

                # software pipeline: scores/exp/mask run LOOKAHEAD iterations
                # ahead of the attnV accumulation so PE never waits in-line.
                LOOKAHEAD = 1
                for hp in range(HPC // 2):  # head pairs
                    out_ps = [
                        [
                            opp.tile([128, 512], F32, tag="ops", name=f"ops_{hp}_{h2}_{j}")
                            for j in range(2)
                        ]
                        for h2 in range(2)
                    ]  # [h2][j]
                    expm_q = {}
                    for ii in range(NSK + LOOKAHEAD):
                        if ii < NSK:
                            emit_front(hp, ii, expm_q)
                        if ii >= LOOKAHEAD:
                            i = ii - LOOKAHEAD
                            for h2 in range(2):
                                h = 2 * hp + h2
                                expm = expm_q.pop((i, h2))
                                vsl = v_aug[:, i, h * 128 : (h + 1) * 128]
                                for j in range(2):
                                    nc.tensor.matmul(
                                        out_ps[h2][j][:],
                                        vsl,
                                        expm[:, j * 512 : (j + 1) * 512],
                                        start=(i == 0),
                                        stop=(i == NSK - 1),
                                    )
                    # normalize: rows 64..127 of out_ps hold Z replicated;
                    # reciprocal of one row, broadcast on gpsimd, multiply.
                    for h2 in range(2):
                        for j in range(2):
                            zr1 = p2.tile([1, 512], F32, tag="zr1")
                            nc.vector.reciprocal(
                                out=zr1[:], in_=out_ps[h2][j][64:65, :]
                            )
                            zr = p2.tile([64, 512], F32, tag="zr")
                            nc.gpsimd.partition_broadcast(zr[:], zr1[:])
                            nc.vector.tensor_tensor(
                                out=out_cT[
                                    h2 * 64 : (h2 + 1) * 64,
                                    hp,
                                    j * 512 : (j + 1) * 512,
                                ],
                                in0=out_ps[h2][j][0:64, :],
                                in1=zr[:],
                                op=MULT,
                            )

            # ---- phase 3: output projection -----------------------------
            with (
                tc.tile_pool(name="p3", bufs=1) as p3,
                tc.tile_pool(name="p3w", bufs=3) as p3w,
                tc.tile_pool(name="ps3", bufs=4, space="PSUM") as ps3,
            ):
                wo_sb = p3.tile([128, NDB, D], F32R)
                nc.gpsimd.dma_start(out=wo_sb[:], in_=woT_r[:])
                bo_rep = p3.tile([128, D], F32)
                nc.sync.dma_start(
                    out=bo_rep[:], in_=bo.ap()[None, :].to_broadcast((128, D))
                )

                for m in range(SQC // 128):
                    for n in range(D // 512):
                        ps = ps3.tile([128, 512], F32, tag="ps3")
                        for c in range(NDB):
                            nc.tensor.matmul(
                                ps[:],
                                out_cT[:, c, m * 128 : (m + 1) * 128],
                                wo_sb[:, c, n * 512 : (n + 1) * 512],
                                start=(c == 0),
                                stop=(c == NDB - 1),
                            )
                        ob = p3w.tile([128, 512], F32, tag="ob")
                        nc.vector.tensor_tensor(
                            out=ob[:],
                            in0=ps[:],
                            in1=bo_rep[:, n * 512 : (n + 1) * 512],
                            op=ADD,
                        )
                        nc.sync.dma_start(
                            out=outp[m * 128 : (m + 1) * 128, n * 512 : (n + 1) * 512],
                            in_=ob[:],
                        )

    nc.compile()
    return nc


def _get_nc():
    if "nc" not in _CACHE:
        _CACHE["nc"] = _build()
    return _CACHE["nc"]


def _prep_inputs(x, mask, Wq, Wk, Wv, Wo, bo):
    """Build the 8 per-core input maps."""
    x = np.asarray(x, dtype=np.float32)
    mask = np.asarray(mask, dtype=np.int32)
    bo = np.asarray(bo, dtype=np.float32)
    wqT = np.ascontiguousarray(np.asarray(Wq, np.float32).T)
    wkT = np.ascontiguousarray(np.asarray(Wk, np.float32).T)
    wvT = np.ascontiguousarray(np.asarray(Wv, np.float32).T)
    woT = np.ascontiguousarray(np.asarray(Wo, np.float32).T)
    bz = np.zeros_like(bo)

    # The SPMD program always reads q activations from xT columns 0..SQC,
    # so qh==1 cores get xT rolled by -SQC along s (and maskT rows rolled
    # identically).  Attention sums over s_k, so a consistent permutation
    # of the k/V order (with the mask following it) leaves the result
    # unchanged.
    xTs = [np.ascontiguousarray(x[b].T) for b in range(B)]
    xTs_r = [np.ascontiguousarray(np.roll(t, -SQC, axis=1)) for t in xTs]
    maskTs = [np.ascontiguousarray(mask[b, 0].T) for b in range(B)]
    maskTs_r = [np.roll(t, -SQC, axis=0) for t in maskTs]

    in_maps = []
    for c in range(NCORES):
        b, hh, qh = c >> 2, (c >> 1) & 1, c & 1
        doff = hh * DPC
        qoff = qh * SQC
        mT = maskTs[b] if qh == 0 else maskTs_r[b]
        in_maps.append(
            {
                "xT": xTs[b] if qh == 0 else xTs_r[b],
                "wqT": np.ascontiguousarray(wqT[:, doff : doff + DPC]),
                "wkT": np.ascontiguousarray(wkT[:, doff : doff + DPC]),
                "wvT": np.ascontiguousarray(wvT[:, doff : doff + DPC]),
                "woT": np.ascontiguousarray(woT[doff : doff + DPC, :]),
                "maskT": np.ascontiguousarray(mT[:, qoff : qoff + SQC]),
                "bo": bo if hh == 0 else bz,
            }
        )
    return in_maps


def run(inputs: dict, trace: bool = False):
    """Run the kernel; returns (full_output, BassKernelResults)."""
    from concourse.bass_utils import run_bass_kernel_spmd

    nc = _get_nc()
    in_maps = _prep_inputs(**inputs)
    res = run_bass_kernel_spmd(
        nc, in_maps, core_ids=list(range(NCORES)), trace=trace
    )
    out = np.empty((B, S, D), dtype=np.float32)
    for b in range(B):
        for qh in range(2):
            c0 = (b << 2) | (0 << 1) | qh
            c1 = (b << 2) | (1 << 1) | qh
            out[b, qh * SQC : (qh + 1) * SQC, :] = (
                res.results[c0]["outp"] + res.results[c1]["outp"]
            )
    return out, res


def kernel(**inputs) -> np.ndarray:
    out, _ = run(inputs, trace=False)
    return out


# revision 17
# speedup vs baseline: 1.2097x; 1.0173x over previous
"""Multi-head attention (B=2, S=2048, D=1024, H=16, Hd=64) on 8 Trainium2
NeuronCores.

Sharding: 8 cores = (batch 2) x (head-half 2) x (q-half 2).
Core (b, hh, qh) computes, for batch b, heads hh*8..hh*8+8 and query rows
qh*1024..qh*1024+1024, the partial output

    outp = (softmax-attention of its heads restricted to its q rows) @ Wo_part.T
           + bo_part

and the host sums the two head-half partials per (b, qh) block.  bo is fed as
zeros to the hh==1 cores so the bias is counted once.

Device-side layouts (host pre-transposes so every matmul is a natural
lhsT.T @ rhs with the contraction dim on SBUF partitions):
  xT    [D, S]      x[b].T
  wqT/wkT/wvT [D, 512]  W.T column slice for this head-half
  woT   [512, D]    Wo.T row slice for this head-half
  maskT [S, 1024]   mask[b,0].T column slice for this q-half (int32)

Pipeline per core:
  1. qT = (wqT.T @ xT-cols)  [512, 1024],  kT [512, 2048], V [2048, 512]
     all fp32r (full PE speed, ~1e-4 matmul error).
  2. Per head h, per s_k tile i: scoresT tile [128, 1024] = kT_h_i.T @ qT_h
     (K=64; head pairs land on PE row-groups 0-63/64-127 and run
     concurrently), exp on ScalarE (scale=1/8) -> bf16, mask multiply on
     VectorE (bf16, 2x mode), then attnV accumulation
     out_ps [128, 512] += V_aug_i.T @ expm  where V_aug has 64 ones
     columns so PSUM rows 64..127 all hold Z = sum(expm); reciprocal of
     those rows gives 1/Z already replicated across partitions.
  3. out partial [1024, 1024] = out_cT.T @ woT (+ bo broadcast), DMA out.

No collectives; the only cross-core step is the host-side partial sum.
"""

import sys

if "/opt/trn_rl_repo" not in sys.path:
    sys.path.insert(0, "/opt/trn_rl_repo")

import numpy as np

B, S, D = 2, 2048, 1024
H, HD = 16, 64
NCORES = 8
HPC = 8  # heads per core
DPC = HPC * HD  # 512 head dims per core
SQC = S // 2  # 1024 q rows per core
KT = D // 128  # 8 contraction tiles
NSK = S // 128  # 16 s_k tiles
NDB = DPC // 128  # 4 d-blocks of the per-core head dims

_CACHE = {}


def _build():
    import concourse.bacc as bacc
    import concourse.mybir as mybir
    import concourse.tile as tile

    F32 = mybir.dt.float32
    F32R = mybir.dt.float32r
    BF16 = mybir.dt.bfloat16
    I32 = mybir.dt.int32
    MULT = mybir.AluOpType.mult
    ADD = mybir.AluOpType.add
    EQ = mybir.AluOpType.is_equal
    EXP = mybir.ActivationFunctionType.Exp

    nc = bacc.Bacc("TRN2", target_bir_lowering=False, debug=False)

    xT = nc.dram_tensor("xT", [D, S], F32, kind="ExternalInput")
    wqT = nc.dram_tensor("wqT", [D, DPC], F32, kind="ExternalInput")
    wkT = nc.dram_tensor("wkT", [D, DPC], F32, kind="ExternalInput")
    wvT = nc.dram_tensor("wvT", [D, DPC], F32, kind="ExternalInput")
    woT = nc.dram_tensor("woT", [DPC, D], F32, kind="ExternalInput")
    maskT = nc.dram_tensor("maskT", [S, SQC], I32, kind="ExternalInput")
    bo = nc.dram_tensor("bo", [D], F32, kind="ExternalInput")
    outp = nc.dram_tensor("outp", [SQC, D], F32, kind="ExternalOutput")

    xT_r = xT.rearrange("(t p) s -> p t s", p=128)  # [128, KT, S]
    wqT_r = wqT.rearrange("(t p) d -> p t d", p=128)
    wkT_r = wkT.rearrange("(t p) d -> p t d", p=128)
    wvT_r = wvT.rearrange("(t p) d -> p t d", p=128)
    woT_r = woT.rearrange("(c p) d -> p c d", p=128)  # [128, NDB, D]
    maskT_r = maskT.rearrange("(i p) q -> p i q", p=128)  # [128, NSK, SQC]

    with tile.TileContext(nc) as tc:
        with tc.tile_pool(name="keep", bufs=1) as keep:
            # ---- persistent SBUF tensors --------------------------------
            qT_sb = keep.tile([128, NDB, SQC], F32R)  # 16KB/part
            kT_sb = keep.tile([128, NDB, S], F32R)  # 32KB/part
            v_aug = keep.tile([128, NSK, HPC * 128], BF16)  # 32KB/part
            out_cT = keep.tile([128, NDB, SQC], F32R)  # 16KB/part

            # ones block of V_aug (overwritten below on the V columns)
            nc.vector.memset(v_aug[:], 1.0)

            # ---- phase 1: projections (all fp32r) -----------------------
            with (
                tc.tile_pool(name="p1", bufs=1) as p1,
                tc.tile_pool(name="wslot", bufs=2) as wslot,
                tc.tile_pool(name="ps1", bufs=6, space="PSUM") as ps1,
            ):
                x_sb = p1.tile([128, KT, S], F32R)  # 64KB/part
                for t in range(KT):
                    nc.gpsimd.dma_start(out=x_sb[:, t, :], in_=xT_r[:, t, :])

                def load_w(src_r, nm):
                    w = wslot.tile([128, KT, DPC], F32R, tag="w", name=f"w_{nm}")
                    nc.gpsimd.dma_start(out=w[:], in_=src_r[:])
                    return w

                # qT [512, SQC]: lhsT = wqT tile, rhs = xT q-columns
                wq = load_w(wqT_r, "q")
                for db in range(NDB):
                    for j in range(SQC // 512):
                        ps = ps1.tile([128, 512], F32, tag="ps")
                        for t in range(KT):
                            nc.tensor.matmul(
                                ps[:],
                                wq[:, t, db * 128 : (db + 1) * 128],
                                x_sb[:, t, j * 512 : (j + 1) * 512],
                                start=(t == 0),
                                stop=(t == KT - 1),
                            )
                        nc.any.tensor_copy(
                            qT_sb[:, db, j * 512 : (j + 1) * 512], ps[:]
                        )

                # kT [512, S]
                wk = load_w(wkT_r, "k")
                for db in range(NDB):
                    for j in range(S // 512):
                        ps = ps1.tile([128, 512], F32, tag="ps")
                        for t in range(KT):
                            nc.tensor.matmul(
                                ps[:],
                                wk[:, t, db * 128 : (db + 1) * 128],
                                x_sb[:, t, j * 512 : (j + 1) * 512],
                                start=(t == 0),
                                stop=(t == KT - 1),
                            )
                        nc.any.tensor_copy(
                            kT_sb[:, db, j * 512 : (j + 1) * 512], ps[:]
                        )

                # V natural [S, 512]: lhsT = xT s-block (stationary), rhs = wvT
                wv = load_w(wvT_r, "v")
                for sb in range(NSK):
                    ps = ps1.tile([128, 512], F32, tag="ps")
                    for t in range(KT):
                        nc.tensor.matmul(
                            ps[:],
                            x_sb[:, t, sb * 128 : (sb + 1) * 128],
                            wv[:, t, :],
                            start=(t == 0),
                            stop=(t == KT - 1),
                        )
                    # scatter 8 heads' 64 V columns into v_aug head blocks
                    nc.any.tensor_copy(
                        v_aug[:, sb, :]
                        .rearrange("p (h c) -> p h c", h=HPC)[:, :, 0:HD],
                        ps[:].rearrange("p (h c) -> p h c", h=HPC),
                    )

            # ---- phase 2: attention -------------------------------------
            with (
                tc.tile_pool(name="pmask", bufs=1) as pmask,
                tc.tile_pool(name="mstage", bufs=3) as mstage,
                tc.tile_pool(name="p2", bufs=3) as p2,
                tc.tile_pool(name="pexpm", bufs=4) as pexpm,
                tc.tile_pool(name="sc", bufs=3, space="PSUM") as scp,
                tc.tile_pool(name="op", bufs=2, space="PSUM") as opp,
            ):
                # mask conversion: int32 0/1 -> bf16 (mask==0 -> 1.0)
                mask01 = pmask.tile([128, NSK, SQC], BF16)  # 32KB/part
                for i in range(NSK):
                    for half in range(2):
                        sl = slice(half * (SQC // 2), (half + 1) * (SQC // 2))
                        mi = mstage.tile([128, SQC // 2], I32, tag="mi")
                        nc.sync.dma_start(out=mi[:], in_=maskT_r[:, i, sl])
                        nc.vector.tensor_scalar(
                            out=mask01[:, i, sl],
                            in0=mi[:],
                            scalar1=0,
                            scalar2=None,
                            op0=EQ,
                        )

                # software pipeline over i with a (hp, j)-outer structure:
                # per pass, sc needs 2 PSUM banks (bufs=3 -> lookahead) and
                # out_ps 2 banks.  The two heads' score matmuls use PE row
                # groups 0-63 / 64-127 and run concurrently; one ScalarE exp
                # covers both heads.
                LOOKAHEAD = 1
                for hp in range(HPC // 2):  # head pairs
                    for j in range(2):  # s_q half
                        jsl = slice(j * 512, (j + 1) * 512)
                        out_ps = [
                            opp.tile([128, 512], F32, tag="ops", name=f"ops_{hp}_{j}_{h2}")
                            for h2 in range(2)
                        ]
                        expm_q = {}
                        for ii in range(NSK + LOOKAHEAD):
                            if ii < NSK:
                                i = ii
                                sc = scp.tile(
                                    [128, 2, 512], F32, tag="sc", name=f"sc_{hp}_{j}_{i}"
                                )
                                for h2 in range(2):
                                    nc.tensor.matmul(
                                        sc[:, h2, :],
                                        kT_sb[
                                            h2 * 64 : (h2 + 1) * 64,
                                            hp,
                                            i * 128 : (i + 1) * 128,
                                        ],
                                        qT_sb[h2 * 64 : (h2 + 1) * 64, hp, jsl],
                                        start=True,
                                        stop=True,
                                    )
                                expt = p2.tile([128, 2, 512], BF16, tag="expt")
                                nc.scalar.activation(
                                    out=expt[:], in_=sc[:], func=EXP, scale=0.125
                                )
                                expm = pexpm.tile(
                                    [128, 2, 512],
                                    BF16,
                                    tag="expm",
                                    name=f"expm_{hp}_{j}_{i}",
                                )
                                for h2 in range(2):
                                    nc.vector.tensor_tensor(
                                        out=expm[:, h2, :],
                                        in0=expt[:, h2, :],
                                        in1=mask01[:, i, jsl],
                                        op=MULT,
                                    )
                                expm_q[i] = expm
                            if ii >= LOOKAHEAD:
                                i = ii - LOOKAHEAD
                                expm = expm_q.pop(i)
                                for h2 in range(2):
                                    h = 2 * hp + h2
                                    nc.tensor.matmul(
                                        out_ps[h2][:],
                                        v_aug[:, i, h * 128 : (h + 1) * 128],
                                        expm[:, h2, :],
                                        start=(i == 0),
                                        stop=(i == NSK - 1),
                                    )
                        # normalize: rows 64..127 of out_ps hold Z replicated;
                        # cheap approx reciprocal of one row, broadcast on
                        # gpsimd, multiply into out_cT.
                        for h2 in range(2):
                            zrow = p2.tile([1, 512], F32, tag="zrow")
                            nc.vector.tensor_copy(zrow[:], out_ps[h2][64:65, :])
                            zr1 = p2.tile([1, 512], F32, tag="zr1")
                            nc.vector.reciprocal_approx_fast(
                                out=zr1[:], in_=zrow[:]
                            )
                            zr = p2.tile([64, 512], F32, tag="zr")
                            nc.gpsimd.partition_broadcast(zr[:], zr1[:])
                            nc.vector.tensor_tensor(
                                out=out_cT[h2 * 64 : (h2 + 1) * 64, hp, jsl],
                                in0=out_ps[h2][0:64, :],
                                in1=zr[:],
                                op=MULT,
                            )

            # ---- phase 3: output projection -----------------------------
            with (
                tc.tile_pool(name="p3", bufs=1) as p3,
                tc.tile_pool(name="p3w", bufs=3) as p3w,
                tc.tile_pool(name="ps3", bufs=4, space="PSUM") as ps3,
            ):
                wo_sb = p3.tile([128, NDB, D], F32R)
                nc.gpsimd.dma_start(out=wo_sb[:], in_=woT_r[:])
                bo_rep = p3.tile([128, D], F32)
                nc.sync.dma_start(
                    out=bo_rep[:], in_=bo.ap()[None, :].to_broadcast((128, D))
                )

                for m in range(SQC // 128):
                    for n in range(D // 512):
                        ps = ps3.tile([128, 512], F32, tag="ps3")
                        for c in range(NDB):
                            nc.tensor.matmul(
                                ps[:],
                                out_cT[:, c, m * 128 : (m + 1) * 128],
                                wo_sb[:, c, n * 512 : (n + 1) * 512],
                                start=(c == 0),
                                stop=(c == NDB - 1),
                            )
                        ob = p3w.tile([128, 512], F32, tag="ob")
                        nc.vector.tensor_tensor(
                            out=ob[:],
                            in0=ps[:],
                            in1=bo_rep[:, n * 512 : (n + 1) * 512],
                            op=ADD,
                        )
                        nc.sync.dma_start(
                            out=outp[m * 128 : (m + 1) * 128, n * 512 : (n + 1) * 512],
                            in_=ob[:],
                        )

    nc.compile()
    return nc


def _get_nc():
    if "nc" not in _CACHE:
        _CACHE["nc"] = _build()
    return _CACHE["nc"]


def _prep_inputs(x, mask, Wq, Wk, Wv, Wo, bo):
    """Build the 8 per-core input maps."""
    x = np.asarray(x, dtype=np.float32)
    mask = np.asarray(mask, dtype=np.int32)
    bo = np.asarray(bo, dtype=np.float32)
    wqT = np.ascontiguousarray(np.asarray(Wq, np.float32).T)
    wkT = np.ascontiguousarray(np.asarray(Wk, np.float32).T)
    wvT = np.ascontiguousarray(np.asarray(Wv, np.float32).T)
    woT = np.ascontiguousarray(np.asarray(Wo, np.float32).T)
    bz = np.zeros_like(bo)

    # The SPMD program always reads q activations from xT columns 0..SQC,
    # so qh==1 cores get xT rolled by -SQC along s (and maskT rows rolled
    # identically).  Attention sums over s_k, so a consistent permutation
    # of the k/V order (with the mask following it) leaves the result
    # unchanged.
    xTs = [np.ascontiguousarray(x[b].T) for b in range(B)]
    xTs_r = [np.ascontiguousarray(np.roll(t, -SQC, axis=1)) for t in xTs]
    maskTs = [np.ascontiguousarray(mask[b, 0].T) for b in range(B)]
    maskTs_r = [np.roll(t, -SQC, axis=0) for t in maskTs]

    in_maps = []
    for c in range(NCORES):
        b, hh, qh = c >> 2, (c >> 1) & 1, c & 1
        doff = hh * DPC
        qoff = qh * SQC
        mT = maskTs[b] if qh == 0 else maskTs_r[b]
        in_maps.append(
            {
                "xT": xTs[b] if qh == 0 else xTs_r[b],
                "wqT": np.ascontiguousarray(wqT[:, doff : doff + DPC]),
                "wkT": np.ascontiguousarray(wkT[:, doff : doff + DPC]),
                "wvT": np.ascontiguousarray(wvT[:, doff : doff + DPC]),
                "woT": np.ascontiguousarray(woT[doff : doff + DPC, :]),
                "maskT": np.ascontiguousarray(mT[:, qoff : qoff + SQC]),
                "bo": bo if hh == 0 else bz,
            }
        )
    return in_maps


def run(inputs: dict, trace: bool = False):
    """Run the kernel; returns (full_output, BassKernelResults)."""
    from concourse.bass_utils import run_bass_kernel_spmd

    nc = _get_nc()
    in_maps = _prep_inputs(**inputs)
    res = run_bass_kernel_spmd(
        nc, in_maps, core_ids=list(range(NCORES)), trace=trace
    )
    out = np.empty((B, S, D), dtype=np.float32)
    for b in range(B):
        for qh in range(2):
            c0 = (b << 2) | (0 << 1) | qh
            c1 = (b << 2) | (1 << 1) | qh
            out[b, qh * SQC : (qh + 1) * SQC, :] = (
                res.results[c0]["outp"] + res.results[c1]["outp"]
            )
    return out, res


def kernel(**inputs) -> np.ndarray:
    out, _ = run(inputs, trace=False)
    return out


# revision 27
# speedup vs baseline: 1.2123x; 1.0022x over previous
"""Multi-head attention (B=2, S=2048, D=1024, H=16, Hd=64) on 8 Trainium2
NeuronCores.

Sharding: 8 cores = (batch 2) x (head-half 2) x (q-half 2).
Core (b, hh, qh) computes, for batch b, heads hh*8..hh*8+8 and query rows
qh*1024..qh*1024+1024, the partial output

    outp = (softmax-attention of its heads restricted to its q rows) @ Wo_part.T
           + bo_part

and the host sums the two head-half partials per (b, qh) block.  bo is fed as
zeros to the hh==1 cores so the bias is counted once.

Device-side layouts (host pre-transposes so every matmul is a natural
lhsT.T @ rhs with the contraction dim on SBUF partitions):
  xT    [D, S]      x[b].T
  wqT/wkT/wvT [D, 512]  W.T column slice for this head-half
  woT   [512, D]    Wo.T row slice for this head-half
  maskT [S, 1024]   mask[b,0].T column slice for this q-half (int32)

Pipeline per core:
  1. qT = (wqT.T @ xT-cols)  [512, 1024],  kT [512, 2048], V [2048, 512]
     all fp32r (full PE speed, ~1e-4 matmul error).
  2. Per head h, per s_k tile i: scoresT tile [128, 1024] = kT_h_i.T @ qT_h
     (K=64; head pairs land on PE row-groups 0-63/64-127 and run
     concurrently), exp on ScalarE (scale=1/8) -> bf16, mask multiply on
     VectorE (bf16, 2x mode), then attnV accumulation
     out_ps [128, 512] += V_aug_i.T @ expm  where V_aug has 64 ones
     columns so PSUM rows 64..127 all hold Z = sum(expm); reciprocal of
     those rows gives 1/Z already replicated across partitions.
  3. out partial [1024, 1024] = out_cT.T @ woT (+ bo broadcast), DMA out.

No collectives; the only cross-core step is the host-side partial sum.
"""

import sys

if "/opt/trn_rl_repo" not in sys.path:
    sys.path.insert(0, "/opt/trn_rl_repo")

import numpy as np

B, S, D = 2, 2048, 1024
H, HD = 16, 64
NCORES = 8
HPC = 8  # heads per core
DPC = HPC * HD  # 512 head dims per core
SQC = S // 2  # 1024 q rows per core
KT = D // 128  # 8 contraction tiles
NSK = S // 128  # 16 s_k tiles
NDB = DPC // 128  # 4 d-blocks of the per-core head dims

_CACHE = {}


def _build():
    import concourse.bacc as bacc
    import concourse.mybir as mybir
    import concourse.tile as tile

    F32 = mybir.dt.float32
    F32R = mybir.dt.float32r
    BF16 = mybir.dt.bfloat16
    I32 = mybir.dt.int32
    MULT = mybir.AluOpType.mult
    ADD = mybir.AluOpType.add
    EQ = mybir.AluOpType.is_equal
    EXP = mybir.ActivationFunctionType.Exp

    nc = bacc.Bacc("TRN2", target_bir_lowering=False, debug=False)

    xT = nc.dram_tensor("xT", [D, S], F32, kind="ExternalInput")
    wqT = nc.dram_tensor("wqT", [D, DPC], F32, kind="ExternalInput")
    wkT = nc.dram_tensor("wkT", [D, DPC], F32, kind="ExternalInput")
    wvT = nc.dram_tensor("wvT", [D, DPC], F32, kind="ExternalInput")
    woT = nc.dram_tensor("woT", [DPC, D], F32, kind="ExternalInput")
    maskT = nc.dram_tensor("maskT", [S, SQC], I32, kind="ExternalInput")
    bo = nc.dram_tensor("bo", [D], F32, kind="ExternalInput")
    outp = nc.dram_tensor("outp", [SQC, D], F32, kind="ExternalOutput")

    xT_r = xT.rearrange("(t p) s -> p t s", p=128)  # [128, KT, S]
    wqT_r = wqT.rearrange("(t p) d -> p t d", p=128)
    wkT_r = wkT.rearrange("(t p) d -> p t d", p=128)
    wvT_r = wvT.rearrange("(t p) d -> p t d", p=128)
    woT_r = woT.rearrange("(c p) d -> p c d", p=128)  # [128, NDB, D]
    maskT_r = maskT.rearrange("(i p) q -> p i q", p=128)  # [128, NSK, SQC]

    NM_KEEP = 3  # mask tiles convertible during phase 1

    with tile.TileContext(nc) as tc:
        with (
            tc.tile_pool(name="keep", bufs=1) as keep,
            tc.tile_pool(name="mstage", bufs=4) as mstage_keep,
        ):
            # ---- persistent SBUF tensors --------------------------------
            qT_sb = keep.tile([128, NDB, SQC], F32R)  # 16KB/part
            kT_sb = keep.tile([128, NDB, S], F32R)  # 32KB/part
            v_aug = keep.tile([128, NSK, HPC * 128], BF16)  # 32KB/part
            out_cT = keep.tile([128, NDB, SQC], F32R)  # 16KB/part

            # ones block of V_aug (overwritten below on the V columns)
            nc.vector.memset(v_aug[:], 1.0)

            # mask conversion pipeline: int32 0/1 -> bf16 (mask==0 -> 1.0).
            # Separate tile per s_k block so consumers start as soon as
            # their block is converted; the first NM_KEEP live in this pool
            # (addresses disjoint from phase 1) so they convert early.
            mask01 = [None] * NSK

            def emit_mask(pool, i):
                m = pool.tile([128, SQC], BF16, tag=f"m{i}", name=f"mask01_{i}")
                for half in range(2):
                    sl = slice(half * (SQC // 2), (half + 1) * (SQC // 2))
                    mi = mstage_keep.tile([128, SQC // 2], I32, tag="mi")
                    nc.sync.dma_start(out=mi[:], in_=maskT_r[:, i, sl])
                    nc.vector.tensor_scalar(
                        out=m[:, sl],
                        in0=mi[:],
                        scalar1=0,
                        scalar2=None,
                        op0=EQ,
                    )
                mask01[i] = m

            for i in range(NM_KEEP):
                emit_mask(keep, i)

            # ---- phase 1: projections (all fp32r) -----------------------
            with (
                tc.tile_pool(name="p1", bufs=1) as p1,
                tc.tile_pool(name="wslot", bufs=1) as wslot,
                tc.tile_pool(name="stg", bufs=2) as stg,
                tc.tile_pool(name="ps1", bufs=6, space="PSUM") as ps1,
            ):
                # x and W via fast HWDGE fp32 DMA into staging, converted to
                # fp32r by the otherwise-idle DVE/ACT engines.
                x_sb = p1.tile([128, KT, S], F32R)  # 64KB/part
                _flip = [0]

                def stage_convert(dram_ap, dst_ap):
                    st = stg.tile([128, S], F32, tag="xs")
                    sz = 1
                    for d in dram_ap.shape[1:]:
                        sz *= d
                    src = st[:, 0:sz]
                    if len(dst_ap.shape) == 3:
                        src = src.rearrange(
                            "p (a b) -> p a b", b=dst_ap.shape[2]
                        )
                    nc.sync.dma_start(out=st[:, 0:sz], in_=dram_ap)
                    _flip[0] ^= 1
                    if _flip[0]:
                        nc.vector.tensor_copy(dst_ap, src)
                    else:
                        nc.scalar.copy(dst_ap, src)

                def load_w(src_r, nm):
                    w = wslot.tile([128, KT, DPC], F32R, tag="w", name=f"w_{nm}")
                    hk = KT // 2
                    for h in range(2):
                        stage_convert(
                            src_r[:, h * hk : (h + 1) * hk, :],
                            w[:, h * hk : (h + 1) * hk, :],
                        )
                    return w

                for t in range(KT):
                    stage_convert(xT_r[:, t, :], x_sb[:, t, :])

                # qT [512, SQC]: lhsT = wqT tile, rhs = xT q-columns
                wq = load_w(wqT_r, "q")
                for db in range(NDB):
                    for j in range(SQC // 512):
                        ps = ps1.tile([128, 512], F32, tag="ps")
                        for t in range(KT):
                            nc.tensor.matmul(
                                ps[:],
                                wq[:, t, db * 128 : (db + 1) * 128],
                                x_sb[:, t, j * 512 : (j + 1) * 512],
                                start=(t == 0),
                                stop=(t == KT - 1),
                            )
                        nc.any.tensor_copy(
                            qT_sb[:, db, j * 512 : (j + 1) * 512], ps[:]
                        )

                # kT [512, S]
                wk = load_w(wkT_r, "k")
                for db in range(NDB):
                    for j in range(S // 512):
                        ps = ps1.tile([128, 512], F32, tag="ps")
                        for t in range(KT):
                            nc.tensor.matmul(
                                ps[:],
                                wk[:, t, db * 128 : (db + 1) * 128],
                                x_sb[:, t, j * 512 : (j + 1) * 512],
                                start=(t == 0),
                                stop=(t == KT - 1),
                            )
                        nc.any.tensor_copy(
                            kT_sb[:, db, j * 512 : (j + 1) * 512], ps[:]
                        )

                # V natural [S, 512]: lhsT = xT s-block (stationary), rhs = wvT
                wv = load_w(wvT_r, "v")
                for sb in range(NSK):
                    ps = ps1.tile([128, 512], F32, tag="ps")
                    for t in range(KT):
                        nc.tensor.matmul(
                            ps[:],
                            x_sb[:, t, sb * 128 : (sb + 1) * 128],
                            wv[:, t, :],
                            start=(t == 0),
                            stop=(t == KT - 1),
                        )
                    # scatter 8 heads' 64 V columns into v_aug head blocks
                    nc.any.tensor_copy(
                        v_aug[:, sb, :]
                        .rearrange("p (h c) -> p h c", h=HPC)[:, :, 0:HD],
                        ps[:].rearrange("p (h c) -> p h c", h=HPC),
                    )

            # ---- phase 2: attention -------------------------------------
            with (
                tc.tile_pool(name="pmask", bufs=1) as pmask,
                tc.tile_pool(name="p2", bufs=3) as p2,
                tc.tile_pool(name="pexpm", bufs=4) as pexpm,
                tc.tile_pool(name="sc", bufs=3, space="PSUM") as scp,
                tc.tile_pool(name="op", bufs=2, space="PSUM") as opp,
            ):
                for i in range(NM_KEEP, NSK):
                    emit_mask(pmask, i)

                # software pipeline over i with a (hp, j)-outer structure:
                # per pass, sc needs 2 PSUM banks (bufs=3 -> lookahead) and
                # out_ps 2 banks.  The two heads' score matmuls use PE row
                # groups 0-63 / 64-127 and run concurrently; one ScalarE exp
                # covers both heads.
                LOOKAHEAD = 1
                for hp in range(HPC // 2):  # head pairs
                    for j in range(2):  # s_q half
                        jsl = slice(j * 512, (j + 1) * 512)
                        out_ps = [
                            opp.tile([128, 512], F32, tag="ops", name=f"ops_{hp}_{j}_{h2}")
                            for h2 in range(2)
                        ]
                        expm_q = {}
                        for ii in range(NSK + LOOKAHEAD):
                            if ii < NSK:
                                i = ii
                                sc = scp.tile(
                                    [128, 2, 512], F32, tag="sc", name=f"sc_{hp}_{j}_{i}"
                                )
                                for h2 in range(2):
                                    nc.tensor.matmul(
                                        sc[:, h2, :],
                                        kT_sb[
                                            h2 * 64 : (h2 + 1) * 64,
                                            hp,
                                            i * 128 : (i + 1) * 128,
                                        ],
                                        qT_sb[h2 * 64 : (h2 + 1) * 64, hp, jsl],
                                        start=True,
                                        stop=True,
                                    )
                                expt = p2.tile([128, 2, 512], BF16, tag="expt")
                                nc.scalar.activation(
                                    out=expt[:], in_=sc[:], func=EXP, scale=0.125
                                )
                                expm = pexpm.tile(
                                    [128, 2, 512],
                                    BF16,
                                    tag="expm",
                                    name=f"expm_{hp}_{j}_{i}",
                                )
                                for h2 in range(2):
                                    nc.vector.tensor_tensor(
                                        out=expm[:, h2, :],
                                        in0=expt[:, h2, :],
                                        in1=mask01[i][:, jsl],
                                        op=MULT,
                                    )
                                expm_q[i] = expm
                            if ii >= LOOKAHEAD:
                                i = ii - LOOKAHEAD
                                expm = expm_q.pop(i)
                                for h2 in range(2):
                                    h = 2 * hp + h2
                                    nc.tensor.matmul(
                                        out_ps[h2][:],
                                        v_aug[:, i, h * 128 : (h + 1) * 128],
                                        expm[:, h2, :],
                                        start=(i == 0),
                                        stop=(i == NSK - 1),
                                    )
                        # normalize: rows 64..127 of out_ps hold Z replicated;
                        # cheap approx reciprocal of one row, broadcast on
                        # gpsimd, multiply into out_cT.
                        for h2 in range(2):
                            zrow = p2.tile([1, 512], F32, tag="zrow")
                            nc.vector.tensor_copy(zrow[:], out_ps[h2][64:65, :])
                            zr1 = p2.tile([1, 512], F32, tag="zr1")
                            nc.vector.reciprocal_approx_fast(
                                out=zr1[:], in_=zrow[:]
                            )
                            zr = p2.tile([64, 512], F32, tag="zr")
                            nc.gpsimd.partition_broadcast(zr[:], zr1[:])
                            nc.vector.tensor_tensor(
                                out=out_cT[h2 * 64 : (h2 + 1) * 64, hp, jsl],
                                in0=out_ps[h2][0:64, :],
                                in1=zr[:],
                                op=MULT,
                            )

            # ---- phase 3: output projection -----------------------------
            with (
                tc.tile_pool(name="p3", bufs=1) as p3,
                tc.tile_pool(name="p3w", bufs=3) as p3w,
                tc.tile_pool(name="ps3", bufs=4, space="PSUM") as ps3,
            ):
                wo_sb = p3.tile([128, NDB, D], F32R)
                nc.gpsimd.dma_start(out=wo_sb[:], in_=woT_r[:])
                bo_rep = p3.tile([128, D], F32)
                nc.sync.dma_start(
                    out=bo_rep[:], in_=bo.ap()[None, :].to_broadcast((128, D))
                )

                for m in range(SQC // 128):
                    for n in range(D // 512):
                        ps = ps3.tile([128, 512], F32, tag="ps3")
                        for c in range(NDB):
                            nc.tensor.matmul(
                                ps[:],
                                out_cT[:, c, m * 128 : (m + 1) * 128],
                                wo_sb[:, c, n * 512 : (n + 1) * 512],
                                start=(c == 0),
                                stop=(c == NDB - 1),
                            )
                        ob = p3w.tile([128, 512], F32, tag="ob")
                        nc.vector.tensor_tensor(
                            out=ob[:],
                            in0=ps[:],
                            in1=bo_rep[:, n * 512 : (n + 1) * 512],
                            op=ADD,
                        )
                        nc.sync.dma_start(
                            out=outp[m * 128 : (m + 1) * 128, n * 512 : (n + 1) * 512],
                            in_=ob[:],
                        )

    nc.compile()
    return nc


def _get_nc():
    if "nc" not in _CACHE:
        _CACHE["nc"] = _build()
    return _CACHE["nc"]


def _prep_inputs(x, mask, Wq, Wk, Wv, Wo, bo):
    """Build the 8 per-core input maps."""
    x = np.asarray(x, dtype=np.float32)
    mask = np.asarray(mask, dtype=np.int32)
    bo = np.asarray(bo, dtype=np.float32)
    wqT = np.ascontiguousarray(np.asarray(Wq, np.float32).T)
    wkT = np.ascontiguousarray(np.asarray(Wk, np.float32).T)
    wvT = np.ascontiguousarray(np.asarray(Wv, np.float32).T)
    woT = np.ascontiguousarray(np.asarray(Wo, np.float32).T)
    bz = np.zeros_like(bo)

    # The SPMD program always reads q activations from xT columns 0..SQC,
    # so qh==1 cores get xT rolled by -SQC along s (and maskT rows rolled
    # identically).  Attention sums over s_k, so a consistent permutation
    # of the k/V order (with the mask following it) leaves the result
    # unchanged.
    xTs = [np.ascontiguousarray(x[b].T) for b in range(B)]
    xTs_r = [np.ascontiguousarray(np.roll(t, -SQC, axis=1)) for t in xTs]
    maskTs = [np.ascontiguousarray(mask[b, 0].T) for b in range(B)]
    maskTs_r = [np.roll(t, -SQC, axis=0) for t in maskTs]

    in_maps = []
    for c in range(NCORES):
        b, hh, qh = c >> 2, (c >> 1) & 1, c & 1
        doff = hh * DPC
        qoff = qh * SQC
        mT = maskTs[b] if qh == 0 else maskTs_r[b]
        in_maps.append(
            {
                "xT": xTs[b] if qh == 0 else xTs_r[b],
                "wqT": np.ascontiguousarray(wqT[:, doff : doff + DPC]),
                "wkT": np.ascontiguousarray(wkT[:, doff : doff + DPC]),
                "wvT": np.ascontiguousarray(wvT[:, doff : doff + DPC]),
                "woT": np.ascontiguousarray(woT[doff : doff + DPC, :]),
                "maskT": np.ascontiguousarray(mT[:, qoff : qoff + SQC]),
                "bo": bo if hh == 0 else bz,
            }
        )
    return in_maps


def run(inputs: dict, trace: bool = False):
    """Run the kernel; returns (full_output, BassKernelResults)."""
    from concourse.bass_utils import run_bass_kernel_spmd

    nc = _get_nc()
    in_maps = _prep_inputs(**inputs)
    res = run_bass_kernel_spmd(
        nc, in_maps, core_ids=list(range(NCORES)), trace=trace
    )
    out = np.empty((B, S, D), dtype=np.float32)
    for b in range(B):
        for qh in range(2):
            c0 = (b << 2) | (0 << 1) | qh
            c1 = (b << 2) | (1 << 1) | qh
            out[b, qh * SQC : (qh + 1) * SQC, :] = (
                res.results[c0]["outp"] + res.results[c1]["outp"]
            )
    return out, res


def kernel(**inputs) -> np.ndarray:
    out, _ = run(inputs, trace=False)
    return out
